# revision 30
# baseline (speedup 1.0000x reference)
"""ComplexAttentionV3 Trainium2 kernel (v3).

Sharding: 8 cores = data-parallel over batch (2) x tensor-parallel over
heads (16 -> 4 per core). Each core computes q/k/v for its 4 heads
(column-sharded projections), local attention, and a row-sharded
o-projection producing a partial [T, D] output; the host sums the 4
partials per batch.

v9 notes vs v2 (559us baseline; this version ~409us):
- softmax denominator no longer uses 256 ones-matmuls on the PE (55us
  of pure streaming overhead + LDWEIGHTS thrash between av and dn);
  exp tiles are accumulated on the DVE in bf16 and reduced with 2 tiny
  ones-matmuls per (head, window). Attention is now bound by the
  scalar engine's exp stream (128 x 1114ns), which runs saturated.
- each window's softmax finisher (dn -> rec -> broadcast -> muls) is a
  serial cross-engine chain; its pieces are emitted a few jc
  iterations INTO the next window so the chain pipelines under exp.
- the gpsimd partition_broadcast program is warmed up at kernel start:
  its first dispatch costs ~7.5us and otherwise lands mid-attention,
  chaining into a full-pipeline stall.
- attention av PSUM drains to SBUF via a vector copy so the
  accumulator bank frees early; normalization runs off-PSUM.
- x lands in 512-col pieces ordered by first use, xr on the SP queue
  and xi on the ACT queue (one queue cannot feed the qk phase);
  cos/sin tables are bf16 and slot between xr quarters just ahead of
  their RoPE drains. First matmul starts at ~3us instead of ~36us.
- o-projection weights prefetch on the SP queue during the qk phase;
  outputs are written as bf16 split across both DMA queues in 512-col
  halves (a single queue is bandwidth-bound on 8MB of output), summed
  in f32 on the host. v-proj and o-proj PSUM drain copies run on the
  vector engine, keeping scalar free for exp.
- PSUM pools: qk uses all 8 banks double-buffered; v-projection and
  attention scores share one 4-bank pool so scores start right after
  the last v matmul; av accumulator 2 banks + dn 2 banks.
"""

import numpy as np
import ml_dtypes

import concourse.bacc as bacc
import concourse.tile as tile
from concourse import mybir
from concourse.bass import ts
from concourse.bass_utils import run_bass_kernel_spmd

B, T, D, H = 2, 2048, 1024, 16
HD = 64
NCORE = 8
TP = 4               # head-parallel degree (per batch)
HC = H // TP         # heads per core = 4
C = HC * HD          # local channels = 256
DC = D // 128        # contraction chunks = 8
TQ = T // 128        # 128-row t-chunks = 16
TW = T // 1024       # 1024-col t-chunks = 2

F32 = mybir.dt.float32
BF16 = mybir.dt.bfloat16
EXP = mybir.ActivationFunctionType.Exp

LAST_RESULTS = None
_COMPILED = None


def _build():
    nc = bacc.Bacc("TRN2", target_bir_lowering=False, debug=False,
                   num_devices=NCORE)

    def din(name, shape, dt=BF16):
        return nc.dram_tensor(name, shape, dt, kind="ExternalInput").ap()

    xr_d = din("xrT", [128, DC, T])
    xi_d = din("xiT", [128, DC, T])
    wq = {k: din(f"wq_{k}", [128, DC, C]) for k in ("r", "i", "n")}
    wk = {k: din(f"wk_{k}", [128, DC, C]) for k in ("r", "i", "n")}
    wv = {k: din(f"wv_{k}", [128, DC, 2 * C]) for k in ("a", "b")}
    ow = {k: din(f"ow_{k}", [128, 2, D]) for k in ("r", "i", "n")}
    cos_d = din("cos2", [128, T], BF16)
    sin_d = din("sin2", [128, T], BF16)
    outr_d = nc.dram_tensor("out_r", [T, D], BF16, kind="ExternalOutput").ap()
    outi_d = nc.dram_tensor("out_i", [T, D], BF16, kind="ExternalOutput").ap()

    with tile.TileContext(nc) as tc:
        with tc.tile_pool(name="persist", bufs=1) as persist:
            qkcat = persist.tile([128, 2 * HC, T], BF16, name="qkcat")
            vcat = persist.tile([128, TQ, HC, 128], BF16, name="vcat")
            urt = persist.tile([128, 2, T], BF16, name="urt")
            uit = persist.tile([128, 2, T], BF16, name="uit")
            ones = persist.tile([128, 1], BF16, name="ones")
            nc.vector.memset(ones[:], 1.0)
            # dummy broadcast: preloads the gpsimd program while the
            # engine is idle (first dispatch otherwise costs ~7.5us in
            # the middle of the attention phase)
            bwarm_in = persist.tile([1, 8], F32, name="bwarm_in")
            bwarm = persist.tile([128, 8], F32, name="bwarm")
            nc.vector.memset(bwarm_in[:], 1.0)
            nc.gpsimd.partition_broadcast(bwarm[:], bwarm_in[:])

            # -------- input DMA: ordered by first consumer --------
            xw = tc.alloc_tile_pool(name="xw", bufs=1)
            wqs = {k: xw.tile([128, DC, C], BF16, name=f"wq{k}")
                   for k in ("r", "i", "n")}
            wks = {k: xw.tile([128, DC, C], BF16, name=f"wk{k}")
                   for k in ("r", "i", "n")}
            wvs = {k: xw.tile([128, DC, 2 * C], BF16, name=f"wv{k}")
                   for k in ("a", "b")}
            cos = xw.tile([128, T], BF16, name="cos")
            sin = xw.tile([128, T], BF16, name="sin")
            xr = xw.tile([128, DC, T], BF16, name="xr")
            xi = xw.tile([128, DC, T], BF16, name="xi")

            # ACT queue: q weights first (first matmul group), then xi
            # quarters (consumed ~3.5us after the matching xr quarter),
            # rope tables, then k/v weights.
            for k in ("r", "i", "n"):
                nc.scalar.dma_start(wqs[k][:], wq[k][:])
            for q in range(4):
                qs = ts(q, 512)
                for dc in range(DC):
                    nc.scalar.dma_start(xi[:, dc, qs], xi_d[:, dc, qs])
            for k in ("r", "i", "n"):
                nc.scalar.dma_start(wks[k][:], wk[k][:])
            for k in ("a", "b"):
                nc.scalar.dma_start(wvs[k][:], wv[k][:])

            # SP queue: xr in 512-col pieces ordered by first use, with
            # the rope tables slotted in just ahead of their drains and
            # the o-projection weights prefetched at the tail.
            for q in range(2):
                qs = ts(q, 512)
                for dc in range(DC):
                    nc.sync.dma_start(xr[:, dc, qs], xr_d[:, dc, qs])
            nc.sync.dma_start(cos[:, 0:1024], cos_d[:, 0:1024])
            nc.sync.dma_start(sin[:, 0:1024], sin_d[:, 0:1024])
            for q in range(2, 4):
                qs = ts(q, 512)
                for dc in range(DC):
                    nc.sync.dma_start(xr[:, dc, qs], xr_d[:, dc, qs])
            nc.sync.dma_start(cos[:, 1024:2048], cos_d[:, 1024:2048])
            nc.sync.dma_start(sin[:, 1024:2048], sin_d[:, 1024:2048])
            ows = {k: persist.tile([128, 2, D], BF16, name=f"ow{k}")
                   for k in ("r", "i", "n")}
            for k in ("r", "i", "n"):
                nc.sync.dma_start(ows[k][:], ow[k][:])

            # ---------------- q/k projection ----------------
            with tc.tile_pool(name="rt", bufs=1) as rt, \
                 tc.tile_pool(name="pp", bufs=2, space="PSUM") as pp:
                for wsrc, hbase in ((wqs, 0), (wks, HC)):
                    for cc in range(2):
                        h0, h1 = hbase + 2 * cc, hbase + 2 * cc + 1
                        for tw in range(TW):
                            pqr = pp.tile([128, 1024], F32, name="ppa")
                            pqi = pp.tile([128, 1024], F32, name="ppb")
                            for half in range(2):
                                hsl = ts(2 * tw + half, 512)
                                psl = ts(half, 512)
                                for dc in range(DC):
                                    nc.tensor.matmul(
                                        pqr[:, psl],
                                        lhsT=wsrc["r"][:, dc, ts(cc, 128)],
                                        rhs=xr[:, dc, hsl],
                                        start=(dc == 0), stop=False)
                                for dc in range(DC):
                                    nc.tensor.matmul(
                                        pqi[:, psl],
                                        lhsT=wsrc["i"][:, dc, ts(cc, 128)],
                                        rhs=xr[:, dc, hsl],
                                        start=(dc == 0), stop=False)
                                for dc in range(DC):
                                    nc.tensor.matmul(
                                        pqr[:, psl],
                                        lhsT=wsrc["n"][:, dc, ts(cc, 128)],
                                        rhs=xi[:, dc, hsl],
                                        start=False, stop=(dc == DC - 1))
                                for dc in range(DC):
                                    nc.tensor.matmul(
                                        pqi[:, psl],
                                        lhsT=wsrc["r"][:, dc, ts(cc, 128)],
                                        rhs=xi[:, dc, hsl],
                                        start=False, stop=(dc == DC - 1))
                            tsl = ts(tw, 1024)
                            t1 = rt.tile([128, 1024], BF16, name="t1")
                            t2 = rt.tile([128, 1024], BF16, name="t2")
                            t3 = rt.tile([128, 1024], BF16, name="t3")
                            t4 = rt.tile([128, 1024], BF16, name="t4")
                            nc.vector.tensor_mul(t1[:], pqr[:], cos[:, tsl])
                            nc.vector.tensor_mul(t2[:], pqi[:], sin[:, tsl])
                            nc.vector.tensor_mul(t3[:], pqr[:], sin[:, tsl])
                            nc.vector.tensor_mul(t4[:], pqi[:], cos[:, tsl])
                            nc.vector.tensor_sub(qkcat[0:64, h0, tsl],
                                                 t1[0:64, :], t2[0:64, :])
                            nc.vector.tensor_sub(qkcat[0:64, h1, tsl],
                                                 t1[64:128, :], t2[64:128, :])
                            nc.vector.tensor_add(qkcat[64:128, h0, tsl],
                                                 t3[0:64, :], t4[0:64, :])
                            nc.vector.tensor_add(qkcat[64:128, h1, tsl],
                                                 t3[64:128, :], t4[64:128, :])

            # ---------------- v projection ----------------
            vv = tc.alloc_tile_pool(name="vv", bufs=2, space="PSUM")
            # natural [t, c], rhs packed [wvr | wvi]
            for tq in range(TQ):
                pv = vv.tile([128, 1024], F32, name="pv")
                pvs = pv[:, 0:512]
                for dc in range(DC):
                    nc.tensor.matmul(pvs, lhsT=xr[:, dc, ts(tq, 128)],
                                     rhs=wvs["a"][:, dc, :],
                                     start=(dc == 0), stop=False)
                for dc in range(DC):
                    nc.tensor.matmul(pvs, lhsT=xi[:, dc, ts(tq, 128)],
                                     rhs=wvs["b"][:, dc, :],
                                     start=False, stop=(dc == DC - 1))
                nc.vector.tensor_copy(
                    vcat[:, tq, :, 0:64],
                    pv[:, 0:C].rearrange("p (h d) -> p h d", h=HC))
                nc.vector.tensor_copy(
                    vcat[:, tq, :, 64:128],
                    pv[:, C:2 * C].rearrange("p (h d) -> p h d", h=HC))
            vv.release()

            # x and q/k/v weights are consumed; free their SBUF before
            # opening the attention pools.
            xw.release()

            mm = tc.alloc_tile_pool(name="mm", bufs=2, space="PSUM")
            avp = tc.alloc_tile_pool(name="avp", bufs=1, space="PSUM")
            dnp = tc.alloc_tile_pool(name="dnp", bufs=1, space="PSUM")
            att = tc.alloc_tile_pool(name="att", bufs=6)
            asm = tc.alloc_tile_pool(name="asm", bufs=2)

            # ---------------- attention ----------------
            # The per-window softmax finisher (dn -> rec -> bc -> muls)
            # is a serial cross-engine chain; emitting it inline blocks
            # every engine queue at the window boundary. Instead each
            # window's finisher pieces are emitted a few jc iterations
            # INTO the next window so the chain pipelines under exp.
            pend = None  # (esum, avr, dn-slot..) of the previous window

            def fin_dn(p):
                dn = dnp.tile([1, 1024], F32, name="dn")
                for half in range(2):
                    nc.tensor.matmul(dn[:, ts(half, 512)], lhsT=ones[:],
                                     rhs=p["esum"][:, ts(half, 512)],
                                     start=True, stop=True)
                p["dn"] = dn

            def fin_rec(p):
                rec = asm.tile([1, 1024], F32, name="rec")
                nc.vector.reciprocal_approx_fast(rec[:], p["dn"][:])
                p["rec"] = rec

            def fin_bc(p):
                bc = asm.tile([128, 1024], F32, name="bc")
                nc.gpsimd.partition_broadcast(bc[:], p["rec"][:])
                p["bc"] = bc

            def fin_mul(p):
                ucc, up0, isl = p["ucc"], p["up0"], p["isl"]
                nc.vector.tensor_mul(urt[up0:up0 + 64, ucc, isl],
                                     p["avr"][0:64, :], p["bc"][0:64, :])
                nc.vector.tensor_mul(uit[up0:up0 + 64, ucc, isl],
                                     p["avr"][64:128, :], p["bc"][64:128, :])

            for h in range(HC):
                ucc, up0 = h // 2, (h % 2) * 64
                for iw in range(TW):
                    isl = ts(iw, 1024)
                    av = avp.tile([128, 1024], F32, name="av")
                    esum = asm.tile([128, 1024], BF16, name="esum")
                    for jc in range(TQ):
                        s = mm.tile([128, 1024], F32, name="mmt")
                        for half in range(2):
                            nc.tensor.matmul(
                                s[:, ts(half, 512)],
                                lhsT=qkcat[:, HC + h, ts(jc, 128)],
                                rhs=qkcat[:, h, ts(2 * iw + half, 512)],
                                start=True, stop=True)
                        es = att.tile([128, 1024], BF16, name="es")
                        nc.scalar.activation(es[:], s[:], EXP, scale=0.125)
                        for half in range(2):
                            psl = ts(half, 512)
                            nc.tensor.matmul(av[:, psl],
                                             lhsT=vcat[:, jc, h, :],
                                             rhs=es[:, psl],
                                             start=(jc == 0),
                                             stop=(jc == TQ - 1))
                        if jc == 0:
                            nc.vector.tensor_copy(esum[:], es[:])
                        else:
                            nc.vector.tensor_add(esum[:], esum[:], es[:])
                        if pend is not None:
                            if jc == 1:
                                fin_dn(pend)
                            elif jc == 2:
                                fin_rec(pend)
                            elif jc == 3:
                                fin_bc(pend)
                            elif jc == 5:
                                fin_mul(pend)
                                pend = None
                    avr = asm.tile([128, 1024], BF16, name="avr")
                    nc.vector.tensor_copy(avr[:], av[:])
                    pend = {"esum": esum, "avr": avr,
                            "ucc": ucc, "up0": up0, "isl": isl}
            # flush the final window's finisher
            fin_dn(pend)
            fin_rec(pend)
            fin_bc(pend)
            fin_mul(pend)
            pend = None

            asm.release()
            att.release()
            dnp.release()
            avp.release()
            mm.release()

            # ---------------- output projection ----------------
            with tc.tile_pool(name="ost", bufs=3) as ost, \
                 tc.tile_pool(name="op", bufs=2, space="PSUM") as op:
                for tq in range(TQ):
                    tslq = ts(tq, 128)
                    por = op.tile([128, 1024], F32, name="opa")
                    poi = op.tile([128, 1024], F32, name="opb")
                    for oc in range(2):
                        osl = ts(oc, 512)
                        nc.tensor.matmul(por[:, osl], lhsT=urt[:, 0, tslq],
                                         rhs=ows["r"][:, 0, osl],
                                         start=True, stop=False)
                        nc.tensor.matmul(por[:, osl], lhsT=urt[:, 1, tslq],
                                         rhs=ows["r"][:, 1, osl],
                                         start=False, stop=False)
                        nc.tensor.matmul(por[:, osl], lhsT=uit[:, 0, tslq],
                                         rhs=ows["n"][:, 0, osl],
                                         start=False, stop=False)
                        nc.tensor.matmul(por[:, osl], lhsT=uit[:, 1, tslq],
                                         rhs=ows["n"][:, 1, osl],
                                         start=False, stop=True)
                        nc.tensor.matmul(poi[:, osl], lhsT=urt[:, 0, tslq],
                                         rhs=ows["i"][:, 0, osl],
                                         start=True, stop=False)
                        nc.tensor.matmul(poi[:, osl], lhsT=urt[:, 1, tslq],
                                         rhs=ows["i"][:, 1, osl],
                                         start=False, stop=False)
                        nc.tensor.matmul(poi[:, osl], lhsT=uit[:, 0, tslq],
                                         rhs=ows["r"][:, 0, osl],
                                         start=False, stop=False)
                        nc.tensor.matmul(poi[:, osl], lhsT=uit[:, 1, tslq],
                                         rhs=ows["r"][:, 1, osl],
                                         start=False, stop=True)
                    st = ost.tile([128, 1024], BF16, name="st")
                    sti = ost.tile([128, 1024], BF16, name="sti")
                    for oc in range(2):
                        osl = ts(oc, 512)
                        nc.scalar.copy(st[:, osl], por[:, osl])
                        nc.sync.dma_start(outr_d[tslq, osl], st[:, osl])
                    for oc in range(2):
                        osl = ts(oc, 512)
                        nc.vector.tensor_copy(sti[:, osl], poi[:, osl])
                        nc.sync.dma_start(outi_d[tslq, osl], sti[:, osl])

    nc.compile()
    return nc


def _to_bf16_kxm(arr, parts=128):
    """[K, M] fp32 -> [128, K//128, M] bf16 with K split as (chunk, part)."""
    k, m = arr.shape
    out = arr.reshape(k // parts, parts, m).transpose(1, 0, 2)
    return np.ascontiguousarray(out.astype(ml_dtypes.bfloat16))


def _rope_tables():
    inv_freq = 1.0 / (10000.0 ** (np.arange(0, HD, 2, dtype=np.float64) / HD))
    invf64 = np.concatenate([inv_freq, inv_freq])          # [64]
    ang = invf64[:, None] * np.arange(T, dtype=np.float64)[None, :]  # [64, T]
    cos2 = np.tile(np.cos(ang), (2, 1)).astype(ml_dtypes.bfloat16)
    sin2 = np.tile(np.sin(ang), (2, 1)).astype(ml_dtypes.bfloat16)
    return np.ascontiguousarray(cos2), np.ascontiguousarray(sin2)


def kernel(x_real, x_imag, q_wr, q_wi, k_wr, k_wi, v_wr, v_wi, o_wr, o_wi):
    global _COMPILED, LAST_RESULTS
    if _COMPILED is None:
        _COMPILED = _build()
    nc = _COMPILED

    cos2, sin2 = _rope_tables()
    xt = {}
    for b in range(B):
        xt[("r", b)] = _to_bf16_kxm(np.asarray(x_real[b]).T.astype(np.float32))
        xt[("i", b)] = _to_bf16_kxm(np.asarray(x_imag[b]).T.astype(np.float32))

    in_maps = []
    for core in range(NCORE):
        b, g = core // TP, core % TP
        cols = slice(g * C, (g + 1) * C)
        m = {"xrT": xt[("r", b)], "xiT": xt[("i", b)],
             "cos2": cos2, "sin2": sin2}
        for nm, wr_, wi_ in (("wq", q_wr, q_wi), ("wk", k_wr, k_wi)):
            m[f"{nm}_r"] = _to_bf16_kxm(np.asarray(wr_[:, cols]))
            m[f"{nm}_i"] = _to_bf16_kxm(np.asarray(wi_[:, cols]))
            m[f"{nm}_n"] = _to_bf16_kxm(-np.asarray(wi_[:, cols]))
        vr_, vi_ = np.asarray(v_wr[:, cols]), np.asarray(v_wi[:, cols])
        m["wv_a"] = _to_bf16_kxm(np.concatenate([vr_, vi_], axis=1))
        m["wv_b"] = _to_bf16_kxm(np.concatenate([-vi_, vr_], axis=1))
        m["ow_r"] = _to_bf16_kxm(np.asarray(o_wr[cols, :]))
        m["ow_i"] = _to_bf16_kxm(np.asarray(o_wi[cols, :]))
        m["ow_n"] = _to_bf16_kxm(-np.asarray(o_wi[cols, :]))
        in_maps.append(m)

    res = run_bass_kernel_spmd(nc, in_maps, core_ids=list(range(NCORE)))
    LAST_RESULTS = res

    final_r = np.zeros((B, T, D), np.float32)
    final_i = np.zeros((B, T, D), np.float32)
    for core in range(NCORE):
        b = core // TP
        final_r[b] += np.asarray(res.results[core]["out_r"],
                                 dtype=np.float32)
        final_i[b] += np.asarray(res.results[core]["out_i"],
                                 dtype=np.float32)
    return final_r, final_i


# revision 31
# speedup vs baseline: 1.0040x; 1.0040x over previous
"""ComplexAttentionV3 Trainium2 kernel (v3).

Sharding: 8 cores = data-parallel over batch (2) x tensor-parallel over
heads (16 -> 4 per core). Each core computes q/k/v for its 4 heads
(column-sharded projections), local attention, and a row-sharded
o-projection producing a partial [T, D] output; the host sums the 4
partials per batch.

v9 notes vs v2 (559us baseline; this version ~409us):
- softmax denominator no longer uses 256 ones-matmuls on the PE (55us
  of pure streaming overhead + LDWEIGHTS thrash between av and dn);
  exp tiles are accumulated on the DVE in bf16 and reduced with 2 tiny
  ones-matmuls per (head, window). Attention is now bound by the
  scalar engine's exp stream (128 x 1114ns), which runs saturated.
- each window's softmax finisher (dn -> rec -> broadcast -> muls) is a
  serial cross-engine chain; its pieces are emitted a few jc
  iterations INTO the next window so the chain pipelines under exp.
- the gpsimd partition_broadcast program is warmed up at kernel start:
  its first dispatch costs ~7.5us and otherwise lands mid-attention,
  chaining into a full-pipeline stall.
- attention av PSUM drains to SBUF via a vector copy so the
  accumulator bank frees early; normalization runs off-PSUM.
- x lands in 512-col pieces ordered by first use, xr on the SP queue
  and xi on the ACT queue (one queue cannot feed the qk phase);
  cos/sin tables are bf16 and slot between xr quarters just ahead of
  their RoPE drains. First matmul starts at ~3us instead of ~36us.
- o-projection weights prefetch on the SP queue during the qk phase;
  outputs are written as bf16 split across both DMA queues in 512-col
  halves (a single queue is bandwidth-bound on 8MB of output), summed
  in f32 on the host. v-proj and o-proj PSUM drain copies run on the
  vector engine, keeping scalar free for exp.
- PSUM pools: qk uses all 8 banks double-buffered; v-projection and
  attention scores share one 4-bank pool so scores start right after
  the last v matmul; av accumulator 2 banks + dn 2 banks.
"""

import numpy as np
import ml_dtypes

import concourse.bacc as bacc
import concourse.tile as tile
from concourse import mybir
from concourse.bass import ts
from concourse.bass_utils import run_bass_kernel_spmd

B, T, D, H = 2, 2048, 1024, 16
HD = 64
NCORE = 8
TP = 4               # head-parallel degree (per batch)
HC = H // TP         # heads per core = 4
C = HC * HD          # local channels = 256
DC = D // 128        # contraction chunks = 8
TQ = T // 128        # 128-row t-chunks = 16
TW = T // 1024       # 1024-col t-chunks = 2

F32 = mybir.dt.float32
BF16 = mybir.dt.bfloat16
EXP = mybir.ActivationFunctionType.Exp

LAST_RESULTS = None
_COMPILED = None


def _build():
    nc = bacc.Bacc("TRN2", target_bir_lowering=False, debug=False,
                   num_devices=NCORE)

    def din(name, shape, dt=BF16):
        return nc.dram_tensor(name, shape, dt, kind="ExternalInput").ap()

    xr_d = din("xrT", [128, DC, T])
    xi_d = din("xiT", [128, DC, T])
    wq = {k: din(f"wq_{k}", [128, DC, C]) for k in ("r", "i", "n")}
    wk = {k: din(f"wk_{k}", [128, DC, C]) for k in ("r", "i", "n")}
    wv = {k: din(f"wv_{k}", [128, DC, 2 * C]) for k in ("a", "b")}
    ow = {k: din(f"ow_{k}", [128, 2, D]) for k in ("r", "i", "n")}
    cos_d = din("cos2", [128, T], BF16)
    sin_d = din("sin2", [128, T], BF16)
    outr_d = nc.dram_tensor("out_r", [T, D], BF16, kind="ExternalOutput").ap()
    outi_d = nc.dram_tensor("out_i", [T, D], BF16, kind="ExternalOutput").ap()

    with tile.TileContext(nc) as tc:
        with tc.tile_pool(name="persist", bufs=1) as persist:
            qkcat = persist.tile([128, 2 * HC, T], BF16, name="qkcat")
            vcat = persist.tile([128, TQ, HC, 128], BF16, name="vcat")
            urt = persist.tile([128, 2, T], BF16, name="urt")
            uit = persist.tile([128, 2, T], BF16, name="uit")
            ones = persist.tile([128, 1], BF16, name="ones")
            nc.vector.memset(ones[:], 1.0)
            # dummy broadcast: preloads the gpsimd program while the
            # engine is idle (first dispatch otherwise costs ~7.5us in
            # the middle of the attention phase)
            bwarm_in = persist.tile([1, 8], F32, name="bwarm_in")
            bwarm = persist.tile([128, 8], F32, name="bwarm")
            nc.vector.memset(bwarm_in[:], 1.0)
            nc.gpsimd.partition_broadcast(bwarm[:], bwarm_in[:])

            # -------- input DMA: ordered by first consumer --------
            xw = tc.alloc_tile_pool(name="xw", bufs=1)
            wqs = {k: xw.tile([128, DC, C], BF16, name=f"wq{k}")
                   for k in ("r", "i", "n")}
            wks = {k: xw.tile([128, DC, C], BF16, name=f"wk{k}")
                   for k in ("r", "i", "n")}
            wvs = {k: xw.tile([128, DC, 2 * C], BF16, name=f"wv{k}")
                   for k in ("a", "b")}
            cos = xw.tile([128, T], BF16, name="cos")
            sin = xw.tile([128, T], BF16, name="sin")
            xr = xw.tile([128, DC, T], BF16, name="xr")
            xi = xw.tile([128, DC, T], BF16, name="xi")

            # ACT queue: q weights first (first matmul group), then xi
            # quarters (consumed ~3.5us after the matching xr quarter),
            # rope tables, then k/v weights.
            for k in ("r", "i", "n"):
                nc.scalar.dma_start(wqs[k][:], wq[k][:])
            for q in range(4):
                qs = ts(q, 512)
                for dc in range(DC):
                    nc.scalar.dma_start(xi[:, dc, qs], xi_d[:, dc, qs])
            for k in ("r", "i", "n"):
                nc.scalar.dma_start(wks[k][:], wk[k][:])
            for k in ("a", "b"):
                nc.scalar.dma_start(wvs[k][:], wv[k][:])

            # SP queue: xr in 512-col pieces ordered by first use, with
            # the rope tables slotted in just ahead of their drains and
            # the o-projection weights prefetched at the tail.
            for q in range(2):
                qs = ts(q, 512)
                for dc in range(DC):
                    nc.sync.dma_start(xr[:, dc, qs], xr_d[:, dc, qs])
            nc.sync.dma_start(cos[:, 0:1024], cos_d[:, 0:1024])
            nc.sync.dma_start(sin[:, 0:1024], sin_d[:, 0:1024])
            for q in range(2, 4):
                qs = ts(q, 512)
                for dc in range(DC):
                    nc.sync.dma_start(xr[:, dc, qs], xr_d[:, dc, qs])
            nc.sync.dma_start(cos[:, 1024:2048], cos_d[:, 1024:2048])
            nc.sync.dma_start(sin[:, 1024:2048], sin_d[:, 1024:2048])
            ows = {k: persist.tile([128, 2, D], BF16, name=f"ow{k}")
                   for k in ("r", "i", "n")}
            for k in ("r", "i", "n"):
                nc.sync.dma_start(ows[k][:], ow[k][:])

            # ---------------- q/k projection ----------------
            with tc.tile_pool(name="rt", bufs=1) as rt, \
                 tc.tile_pool(name="pp", bufs=2, space="PSUM") as pp:
                for wsrc, hbase in ((wqs, 0), (wks, HC)):
                    for cc in range(2):
                        h0, h1 = hbase + 2 * cc, hbase + 2 * cc + 1
                        for tw in range(TW):
                            pqr = pp.tile([128, 1024], F32, name="ppa")
                            pqi = pp.tile([128, 1024], F32, name="ppb")
                            for half in range(2):
                                hsl = ts(2 * tw + half, 512)
                                psl = ts(half, 512)
                                for dc in range(DC):
                                    nc.tensor.matmul(
                                        pqr[:, psl],
                                        lhsT=wsrc["r"][:, dc, ts(cc, 128)],
                                        rhs=xr[:, dc, hsl],
                                        start=(dc == 0), stop=False)
                                for dc in range(DC):
                                    nc.tensor.matmul(
                                        pqi[:, psl],
                                        lhsT=wsrc["i"][:, dc, ts(cc, 128)],
                                        rhs=xr[:, dc, hsl],
                                        start=(dc == 0), stop=False)
                                for dc in range(DC):
                                    nc.tensor.matmul(
                                        pqr[:, psl],
                                        lhsT=wsrc["n"][:, dc, ts(cc, 128)],
                                        rhs=xi[:, dc, hsl],
                                        start=False, stop=(dc == DC - 1))
                                for dc in range(DC):
                                    nc.tensor.matmul(
                                        pqi[:, psl],
                                        lhsT=wsrc["r"][:, dc, ts(cc, 128)],
                                        rhs=xi[:, dc, hsl],
                                        start=False, stop=(dc == DC - 1))
                            tsl = ts(tw, 1024)
                            t1 = rt.tile([128, 1024], BF16, name="t1")
                            t2 = rt.tile([128, 1024], BF16, name="t2")
                            t3 = rt.tile([128, 1024], BF16, name="t3")
                            t4 = rt.tile([128, 1024], BF16, name="t4")
                            nc.vector.tensor_mul(t1[:], pqr[:], cos[:, tsl])
                            nc.vector.tensor_mul(t2[:], pqi[:], sin[:, tsl])
                            nc.vector.tensor_mul(t3[:], pqr[:], sin[:, tsl])
                            nc.vector.tensor_mul(t4[:], pqi[:], cos[:, tsl])
                            nc.vector.tensor_sub(qkcat[0:64, h0, tsl],
                                                 t1[0:64, :], t2[0:64, :])
                            nc.vector.tensor_sub(qkcat[0:64, h1, tsl],
                                                 t1[64:128, :], t2[64:128, :])
                            nc.vector.tensor_add(qkcat[64:128, h0, tsl],
                                                 t3[0:64, :], t4[0:64, :])
                            nc.vector.tensor_add(qkcat[64:128, h1, tsl],
                                                 t3[64:128, :], t4[64:128, :])

            # ---------------- v projection ----------------
            vv = tc.alloc_tile_pool(name="vv", bufs=2, space="PSUM")
            # natural [t, c], rhs packed [wvr | wvi]
            for tq in range(TQ):
                pv = vv.tile([128, 1024], F32, name="pv")
                pvs = pv[:, 0:512]
                for dc in range(DC):
                    nc.tensor.matmul(pvs, lhsT=xr[:, dc, ts(tq, 128)],
                                     rhs=wvs["a"][:, dc, :],
                                     start=(dc == 0), stop=False)
                for dc in range(DC):
                    nc.tensor.matmul(pvs, lhsT=xi[:, dc, ts(tq, 128)],
                                     rhs=wvs["b"][:, dc, :],
                                     start=False, stop=(dc == DC - 1))
                nc.vector.tensor_copy(
                    vcat[:, tq, :, 0:64],
                    pv[:, 0:C].rearrange("p (h d) -> p h d", h=HC))
                nc.vector.tensor_copy(
                    vcat[:, tq, :, 64:128],
                    pv[:, C:2 * C].rearrange("p (h d) -> p h d", h=HC))
            vv.release()

            # x and q/k/v weights are consumed; free their SBUF before
            # opening the attention pools.
            xw.release()

            mm = tc.alloc_tile_pool(name="mm", bufs=2, space="PSUM")
            avp = tc.alloc_tile_pool(name="avp", bufs=1, space="PSUM")
            dnp = tc.alloc_tile_pool(name="dnp", bufs=1, space="PSUM")
            att = tc.alloc_tile_pool(name="att", bufs=6)
            asm = tc.alloc_tile_pool(name="asm", bufs=2)

            # ---------------- attention ----------------
            # The per-window softmax finisher (dn -> rec -> bc -> muls)
            # is a serial cross-engine chain; emitting it inline blocks
            # every engine queue at the window boundary. Instead each
            # window's finisher pieces are emitted a few jc iterations
            # INTO the next window so the chain pipelines under exp.
            pend = None  # (esum, avr, dn-slot..) of the previous window

            def fin_dn(p):
                dn = dnp.tile([1, 1024], F32, name="dn")
                for half in range(2):
                    nc.tensor.matmul(dn[:, ts(half, 512)], lhsT=ones[:],
                                     rhs=p["esum"][:, ts(half, 512)],
                                     start=True, stop=True)
                p["dn"] = dn

            def fin_rec(p):
                rec = asm.tile([1, 1024], F32, name="rec")
                nc.vector.reciprocal_approx_fast(rec[:], p["dn"][:])
                p["rec"] = rec

            def fin_bc(p):
                bc = asm.tile([128, 1024], F32, name="bc")
                nc.gpsimd.partition_broadcast(bc[:], p["rec"][:])
                p["bc"] = bc

            def fin_mul(p):
                ucc, up0, isl = p["ucc"], p["up0"], p["isl"]
                nc.vector.tensor_mul(urt[up0:up0 + 64, ucc, isl],
                                     p["avr"][0:64, :], p["bc"][0:64, :])
                nc.vector.tensor_mul(uit[up0:up0 + 64, ucc, isl],
                                     p["avr"][64:128, :], p["bc"][64:128, :])

            for h in range(HC):
                ucc, up0 = h // 2, (h % 2) * 64
                for iw in range(TW):
                    isl = ts(iw, 1024)
                    av = avp.tile([128, 1024], F32, name="av")
                    esum = asm.tile([128, 1024], BF16, name="esum")
                    for jc in range(TQ):
                        s = mm.tile([128, 1024], F32, name="mmt")
                        for half in range(2):
                            nc.tensor.matmul(
                                s[:, ts(half, 512)],
                                lhsT=qkcat[:, HC + h, ts(jc, 128)],
                                rhs=qkcat[:, h, ts(2 * iw + half, 512)],
                                start=True, stop=True)
                        es = att.tile([128, 1024], BF16, name="es")
                        nc.scalar.activation(es[:], s[:], EXP, scale=0.125)
                        for half in range(2):
                            psl = ts(half, 512)
                            nc.tensor.matmul(av[:, psl],
                                             lhsT=vcat[:, jc, h, :],
                                             rhs=es[:, psl],
                                             start=(jc == 0),
                                             stop=(jc == TQ - 1))
                        if jc == 0:
                            nc.vector.tensor_copy(esum[:], es[:])
                        else:
                            nc.vector.tensor_add(esum[:], esum[:], es[:])
                        if pend is not None:
                            if jc == 1:
                                fin_dn(pend)
                            elif jc == 2:
                                fin_rec(pend)
                            elif jc == 3:
                                fin_bc(pend)
                            elif jc == 5:
                                fin_mul(pend)
                                pend = None
                    avr = asm.tile([128, 1024], BF16, name="avr")
                    nc.vector.tensor_copy(avr[:], av[:])
                    pend = {"esum": esum, "avr": avr,
                            "ucc": ucc, "up0": up0, "isl": isl}
            # flush the final window's finisher
            fin_dn(pend)
            fin_rec(pend)
            fin_bc(pend)
            fin_mul(pend)
            pend = None

            asm.release()
            att.release()
            dnp.release()
            avp.release()
            mm.release()

            # ---------------- output projection ----------------
            with tc.tile_pool(name="ost", bufs=3) as ost, \
                 tc.tile_pool(name="op", bufs=2, space="PSUM") as op:
                for tq in range(TQ):
                    tslq = ts(tq, 128)
                    por = op.tile([128, 1024], F32, name="opa")
                    poi = op.tile([128, 1024], F32, name="opb")
                    for oc in range(2):
                        osl = ts(oc, 512)
                        nc.tensor.matmul(por[:, osl], lhsT=urt[:, 0, tslq],
                                         rhs=ows["r"][:, 0, osl],
                                         start=True, stop=False)
                        nc.tensor.matmul(por[:, osl], lhsT=urt[:, 1, tslq],
                                         rhs=ows["r"][:, 1, osl],
                                         start=False, stop=False)
                        nc.tensor.matmul(por[:, osl], lhsT=uit[:, 0, tslq],
                                         rhs=ows["n"][:, 0, osl],
                                         start=False, stop=False)
                        nc.tensor.matmul(por[:, osl], lhsT=uit[:, 1, tslq],
                                         rhs=ows["n"][:, 1, osl],
                                         start=False, stop=True)
                        nc.tensor.matmul(poi[:, osl], lhsT=urt[:, 0, tslq],
                                         rhs=ows["i"][:, 0, osl],
                                         start=True, stop=False)
                        nc.tensor.matmul(poi[:, osl], lhsT=urt[:, 1, tslq],
                                         rhs=ows["i"][:, 1, osl],
                                         start=False, stop=False)
                        nc.tensor.matmul(poi[:, osl], lhsT=uit[:, 0, tslq],
                                         rhs=ows["r"][:, 0, osl],
                                         start=False, stop=False)
                        nc.tensor.matmul(poi[:, osl], lhsT=uit[:, 1, tslq],
                                         rhs=ows["r"][:, 1, osl],
                                         start=False, stop=True)
                    st = ost.tile([128, 1024], BF16, name="st")
                    sti = ost.tile([128, 1024], BF16, name="sti")
                    for oc in range(2):
                        osl = ts(oc, 512)
                        nc.scalar.copy(st[:, osl], por[:, osl])
                        nc.sync.dma_start(outr_d[tslq, osl], st[:, osl])
                    # outi on the ACT queue: both outputs on SP alone is
                    # bandwidth-bound (8MB) and drains past the last
                    # matmul.
                    for oc in range(2):
                        osl = ts(oc, 512)
                        nc.vector.tensor_copy(sti[:, osl], poi[:, osl])
                        nc.scalar.dma_start(outi_d[tslq, osl], sti[:, osl])

    nc.compile()
    return nc


def _to_bf16_kxm(arr, parts=128):
    """[K, M] fp32 -> [128, K//128, M] bf16 with K split as (chunk, part)."""
    k, m = arr.shape
    out = arr.reshape(k // parts, parts, m).transpose(1, 0, 2)
    return np.ascontiguousarray(out.astype(ml_dtypes.bfloat16))


def _rope_tables():
    inv_freq = 1.0 / (10000.0 ** (np.arange(0, HD, 2, dtype=np.float64) / HD))
    invf64 = np.concatenate([inv_freq, inv_freq])          # [64]
    ang = invf64[:, None] * np.arange(T, dtype=np.float64)[None, :]  # [64, T]
    cos2 = np.tile(np.cos(ang), (2, 1)).astype(ml_dtypes.bfloat16)
    sin2 = np.tile(np.sin(ang), (2, 1)).astype(ml_dtypes.bfloat16)
    return np.ascontiguousarray(cos2), np.ascontiguousarray(sin2)


def kernel(x_real, x_imag, q_wr, q_wi, k_wr, k_wi, v_wr, v_wi, o_wr, o_wi):
    global _COMPILED, LAST_RESULTS
    if _COMPILED is None:
        _COMPILED = _build()
    nc = _COMPILED

    cos2, sin2 = _rope_tables()
    xt = {}
    for b in range(B):
        xt[("r", b)] = _to_bf16_kxm(np.asarray(x_real[b]).T.astype(np.float32))
        xt[("i", b)] = _to_bf16_kxm(np.asarray(x_imag[b]).T.astype(np.float32))

    in_maps = []
    for core in range(NCORE):
        b, g = core // TP, core % TP
        cols = slice(g * C, (g + 1) * C)
        m = {"xrT": xt[("r", b)], "xiT": xt[("i", b)],
             "cos2": cos2, "sin2": sin2}
        for nm, wr_, wi_ in (("wq", q_wr, q_wi), ("wk", k_wr, k_wi)):
            m[f"{nm}_r"] = _to_bf16_kxm(np.asarray(wr_[:, cols]))
            m[f"{nm}_i"] = _to_bf16_kxm(np.asarray(wi_[:, cols]))
            m[f"{nm}_n"] = _to_bf16_kxm(-np.asarray(wi_[:, cols]))
        vr_, vi_ = np.asarray(v_wr[:, cols]), np.asarray(v_wi[:, cols])
        m["wv_a"] = _to_bf16_kxm(np.concatenate([vr_, vi_], axis=1))
        m["wv_b"] = _to_bf16_kxm(np.concatenate([-vi_, vr_], axis=1))
        m["ow_r"] = _to_bf16_kxm(np.asarray(o_wr[cols, :]))
        m["ow_i"] = _to_bf16_kxm(np.asarray(o_wi[cols, :]))
        m["ow_n"] = _to_bf16_kxm(-np.asarray(o_wi[cols, :]))
        in_maps.append(m)

    res = run_bass_kernel_spmd(nc, in_maps, core_ids=list(range(NCORE)))
    LAST_RESULTS = res

    final_r = np.zeros((B, T, D), np.float32)
    final_i = np.zeros((B, T, D), np.float32)
    for core in range(NCORE):
        b = core // TP
        final_r[b] += np.asarray(res.results[core]["out_r"],
                                 dtype=np.float32)
        final_i[b] += np.asarray(res.results[core]["out_i"],
                                 dtype=np.float32)
    return final_r, final_i


# revision 34
# speedup vs baseline: 1.0250x; 1.0209x over previous
"""ComplexAttentionV3 Trainium2 kernel (v3).

Sharding: 8 cores = data-parallel over batch (2) x tensor-parallel over
heads (16 -> 4 per core). Each core computes q/k/v for its 4 heads
(column-sharded projections), local attention, and a row-sharded
o-projection producing a partial [T, D] output; the host sums the 4
partials per batch.

v9 notes vs v2 (559us baseline; this version ~409us):
- softmax denominator no longer uses 256 ones-matmuls on the PE (55us
  of pure streaming overhead + LDWEIGHTS thrash between av and dn);
  exp tiles are accumulated on the DVE in bf16 and reduced with 2 tiny
  ones-matmuls per (head, window). Attention is now bound by the
  scalar engine's exp stream (128 x 1114ns), which runs saturated.
- each window's softmax finisher (dn -> rec -> broadcast -> muls) is a
  serial cross-engine chain; its pieces are emitted a few jc
  iterations INTO the next window so the chain pipelines under exp.
- the gpsimd partition_broadcast program is warmed up at kernel start:
  its first dispatch costs ~7.5us and otherwise lands mid-attention,
  chaining into a full-pipeline stall.
- attention av PSUM drains to SBUF via a vector copy so the
  accumulator bank frees early; normalization runs off-PSUM.
- x lands in 512-col pieces ordered by first use, xr on the SP queue
  and xi on the ACT queue (one queue cannot feed the qk phase);
  cos/sin tables are bf16 and slot between xr quarters just ahead of
  their RoPE drains. First matmul starts at ~3us instead of ~36us.
- o-projection weights prefetch on the SP queue during the qk phase;
  outputs are written as bf16 split across both DMA queues in 512-col
  halves (a single queue is bandwidth-bound on 8MB of output), summed
  in f32 on the host. v-proj and o-proj PSUM drain copies run on the
  vector engine, keeping scalar free for exp.
- PSUM pools: qk uses all 8 banks double-buffered; v-projection and
  attention scores share one 4-bank pool so scores start right after
  the last v matmul; av accumulator 2 banks + dn 2 banks.
"""

import numpy as np
import ml_dtypes

import concourse.bacc as bacc
import concourse.tile as tile
from concourse import mybir
from concourse.bass import ts
from concourse.bass_utils import run_bass_kernel_spmd

B, T, D, H = 2, 2048, 1024, 16
HD = 64
NCORE = 8
TP = 4               # head-parallel degree (per batch)
HC = H // TP         # heads per core = 4
C = HC * HD          # local channels = 256
DC = D // 128        # contraction chunks = 8
TQ = T // 128        # 128-row t-chunks = 16
TW = T // 1024       # 1024-col t-chunks = 2

F32 = mybir.dt.float32
BF16 = mybir.dt.bfloat16
EXP = mybir.ActivationFunctionType.Exp

LAST_RESULTS = None
_COMPILED = None


def _build():
    nc = bacc.Bacc("TRN2", target_bir_lowering=False, debug=False,
                   num_devices=NCORE)

    def din(name, shape, dt=BF16):
        return nc.dram_tensor(name, shape, dt, kind="ExternalInput").ap()

    xr_d = din("xrT", [128, DC, T])
    xi_d = din("xiT", [128, DC, T])
    wq = {k: din(f"wq_{k}", [128, DC, C]) for k in ("r", "i", "n")}
    wk = {k: din(f"wk_{k}", [128, DC, C]) for k in ("r", "i", "n")}
    wv = {k: din(f"wv_{k}", [128, DC, 2 * C]) for k in ("a", "b")}
    ow = {k: din(f"ow_{k}", [128, 2, D]) for k in ("r", "i", "n")}
    cos_d = din("cos2", [128, T], BF16)
    sin_d = din("sin2", [128, T], BF16)
    outr_d = nc.dram_tensor("out_r", [T, D], BF16, kind="ExternalOutput").ap()
    outi_d = nc.dram_tensor("out_i", [T, D], BF16, kind="ExternalOutput").ap()

    with tile.TileContext(nc) as tc:
        with tc.tile_pool(name="persist", bufs=1) as persist:
            qkcat = persist.tile([128, 2 * HC, T], BF16, name="qkcat")
            vcat = persist.tile([128, TQ, HC, 128], BF16, name="vcat")
            urt = persist.tile([128, 2, T], BF16, name="urt")
            uit = persist.tile([128, 2, T], BF16, name="uit")
            ones = persist.tile([128, 1], BF16, name="ones")
            nc.vector.memset(ones[:], 1.0)
            # dummy broadcast: preloads the gpsimd program while the
            # engine is idle (first dispatch otherwise costs ~7.5us in
            # the middle of the attention phase)
            bwarm_in = persist.tile([1, 8], F32, name="bwarm_in")
            bwarm = persist.tile([128, 8], F32, name="bwarm")
            nc.vector.memset(bwarm_in[:], 1.0)
            nc.gpsimd.partition_broadcast(bwarm[:], bwarm_in[:])

            # -------- input DMA: ordered by first consumer --------
            xw = tc.alloc_tile_pool(name="xw", bufs=1)
            wqs = {k: xw.tile([128, DC, C], BF16, name=f"wq{k}")
                   for k in ("r", "i", "n")}
            wks = {k: xw.tile([128, DC, C], BF16, name=f"wk{k}")
                   for k in ("r", "i", "n")}
            wvs = {k: xw.tile([128, DC, 2 * C], BF16, name=f"wv{k}")
                   for k in ("a", "b")}
            cos = xw.tile([128, T], BF16, name="cos")
            sin = xw.tile([128, T], BF16, name="sin")
            xr = xw.tile([128, DC, T], BF16, name="xr")
            xi = xw.tile([128, DC, T], BF16, name="xi")

            # ACT queue: q weights first (first matmul group), then xi
            # quarters (consumed ~3.5us after the matching xr quarter),
            # rope tables, then k/v weights.
            # wq_r split in dc-halves so the first matmul starts ~1.4us
            # earlier (the dc loop consumes chunks in order).
            nc.scalar.dma_start(wqs["r"][:, 0:4], wq["r"][:, 0:4])
            nc.scalar.dma_start(wqs["r"][:, 4:8], wq["r"][:, 4:8])
            for k in ("i", "n"):
                nc.scalar.dma_start(wqs[k][:], wq[k][:])
            for q in range(4):
                qs = ts(q, 512)
                for dc in range(DC):
                    nc.scalar.dma_start(xi[:, dc, qs], xi_d[:, dc, qs])
            for k in ("r", "i", "n"):
                nc.scalar.dma_start(wks[k][:], wk[k][:])
            for k in ("a", "b"):
                nc.scalar.dma_start(wvs[k][:], wv[k][:])

            # SP queue: xr in 512-col pieces ordered by first use, with
            # the rope tables slotted in just ahead of their drains and
            # the o-projection weights prefetched at the tail.
            for q in range(2):
                qs = ts(q, 512)
                for dc in range(DC):
                    nc.sync.dma_start(xr[:, dc, qs], xr_d[:, dc, qs])
            nc.sync.dma_start(cos[:, 0:1024], cos_d[:, 0:1024])
            nc.sync.dma_start(sin[:, 0:1024], sin_d[:, 0:1024])
            for q in range(2, 4):
                qs = ts(q, 512)
                for dc in range(DC):
                    nc.sync.dma_start(xr[:, dc, qs], xr_d[:, dc, qs])
            nc.sync.dma_start(cos[:, 1024:2048], cos_d[:, 1024:2048])
            nc.sync.dma_start(sin[:, 1024:2048], sin_d[:, 1024:2048])
            ows = {k: persist.tile([128, 2, D], BF16, name=f"ow{k}")
                   for k in ("r", "i", "n")}
            for k in ("r", "i", "n"):
                nc.sync.dma_start(ows[k][:], ow[k][:])

            # ---------------- q/k projection ----------------
            with tc.tile_pool(name="rt", bufs=1) as rt, \
                 tc.tile_pool(name="pp", bufs=2, space="PSUM") as pp:

                def qk_rope(wsrc, hbase, cc, tw, pqr, pqi):
                    h0, h1 = hbase + 2 * cc, hbase + 2 * cc + 1
                    tsl = ts(tw, 1024)
                    t1 = rt.tile([128, 1024], BF16, name="t1")
                    t2 = rt.tile([128, 1024], BF16, name="t2")
                    t3 = rt.tile([128, 1024], BF16, name="t3")
                    t4 = rt.tile([128, 1024], BF16, name="t4")
                    nc.vector.tensor_mul(t1[:], pqr[:], cos[:, tsl])
                    nc.vector.tensor_mul(t2[:], pqi[:], sin[:, tsl])
                    nc.vector.tensor_mul(t3[:], pqr[:], sin[:, tsl])
                    nc.vector.tensor_mul(t4[:], pqi[:], cos[:, tsl])
                    nc.vector.tensor_sub(qkcat[0:64, h0, tsl],
                                         t1[0:64, :], t2[0:64, :])
                    nc.vector.tensor_sub(qkcat[0:64, h1, tsl],
                                         t1[64:128, :], t2[64:128, :])
                    nc.vector.tensor_add(qkcat[64:128, h0, tsl],
                                         t3[0:64, :], t4[0:64, :])
                    nc.vector.tensor_add(qkcat[64:128, h1, tsl],
                                         t3[64:128, :], t4[64:128, :])

                def qk_xr(wsrc, cc, tw, pqr, pqi, half):
                    hsl = ts(2 * tw + half, 512)
                    psl = ts(half, 512)
                    for dc in range(DC):
                        nc.tensor.matmul(pqr[:, psl],
                                         lhsT=wsrc["r"][:, dc, ts(cc, 128)],
                                         rhs=xr[:, dc, hsl],
                                         start=(dc == 0), stop=False)
                    for dc in range(DC):
                        nc.tensor.matmul(pqi[:, psl],
                                         lhsT=wsrc["i"][:, dc, ts(cc, 128)],
                                         rhs=xr[:, dc, hsl],
                                         start=(dc == 0), stop=False)

                def qk_xi(wsrc, cc, tw, pqr, pqi, half):
                    hsl = ts(2 * tw + half, 512)
                    psl = ts(half, 512)
                    for dc in range(DC):
                        nc.tensor.matmul(pqr[:, psl],
                                         lhsT=wsrc["n"][:, dc, ts(cc, 128)],
                                         rhs=xi[:, dc, hsl],
                                         start=False, stop=(dc == DC - 1))
                    for dc in range(DC):
                        nc.tensor.matmul(pqi[:, psl],
                                         lhsT=wsrc["r"][:, dc, ts(cc, 128)],
                                         rhs=xi[:, dc, hsl],
                                         start=False, stop=(dc == DC - 1))

                # Startup special case: the first two q groups (cc0/cc1
                # of tw0) interleave so the PE always has resident-xr
                # work while xi and later x quarters stream in.
                pair = []
                for cc in range(2):
                    pqr = pp.tile([128, 1024], F32, name="ppa")
                    pqi = pp.tile([128, 1024], F32, name="ppb")
                    pair.append((cc, pqr, pqi))
                for half in range(2):
                    for cc, pqr, pqi in pair:
                        qk_xr(wqs, cc, 0, pqr, pqi, half)
                    for cc, pqr, pqi in pair:
                        qk_xi(wqs, cc, 0, pqr, pqi, half)
                for cc, pqr, pqi in pair:
                    qk_rope(wqs, 0, cc, 0, pqr, pqi)

                rest = [(wqs, 0, 0, 1), (wqs, 0, 1, 1)]
                rest += [(wks, HC, cc, tw) for cc in range(2)
                         for tw in range(TW)]
                for wsrc, hbase, cc, tw in rest:
                    pqr = pp.tile([128, 1024], F32, name="ppa")
                    pqi = pp.tile([128, 1024], F32, name="ppb")
                    for half in range(2):
                        qk_xr(wsrc, cc, tw, pqr, pqi, half)
                        qk_xi(wsrc, cc, tw, pqr, pqi, half)
                    qk_rope(wsrc, hbase, cc, tw, pqr, pqi)

            # ---------------- v projection ----------------
            vv = tc.alloc_tile_pool(name="vv", bufs=2, space="PSUM")
            # natural [t, c], rhs packed [wvr | wvi]
            for tq in range(TQ):
                pv = vv.tile([128, 1024], F32, name="pv")
                pvs = pv[:, 0:512]
                for dc in range(DC):
                    nc.tensor.matmul(pvs, lhsT=xr[:, dc, ts(tq, 128)],
                                     rhs=wvs["a"][:, dc, :],
                                     start=(dc == 0), stop=False)
                for dc in range(DC):
                    nc.tensor.matmul(pvs, lhsT=xi[:, dc, ts(tq, 128)],
                                     rhs=wvs["b"][:, dc, :],
                                     start=False, stop=(dc == DC - 1))
                nc.vector.tensor_copy(
                    vcat[:, tq, :, 0:64],
                    pv[:, 0:C].rearrange("p (h d) -> p h d", h=HC))
                nc.vector.tensor_copy(
                    vcat[:, tq, :, 64:128],
                    pv[:, C:2 * C].rearrange("p (h d) -> p h d", h=HC))
            vv.release()

            # x and q/k/v weights are consumed; free their SBUF before
            # opening the attention pools.
            xw.release()

            mm = tc.alloc_tile_pool(name="mm", bufs=2, space="PSUM")
            avp = tc.alloc_tile_pool(name="avp", bufs=1, space="PSUM")
            dnp = tc.alloc_tile_pool(name="dnp", bufs=1, space="PSUM")
            att = tc.alloc_tile_pool(name="att", bufs=6)
            asm = tc.alloc_tile_pool(name="asm", bufs=2)

            # ---------------- attention ----------------
            # The per-window softmax finisher (dn -> rec -> bc -> muls)
            # is a serial cross-engine chain; emitting it inline blocks
            # every engine queue at the window boundary. Instead each
            # window's finisher pieces are emitted a few jc iterations
            # INTO the next window so the chain pipelines under exp.
            pend = None  # (esum, avr, dn-slot..) of the previous window

            def fin_dn(p):
                dn = dnp.tile([1, 1024], F32, name="dn")
                for half in range(2):
                    nc.tensor.matmul(dn[:, ts(half, 512)], lhsT=ones[:],
                                     rhs=p["esum"][:, ts(half, 512)],
                                     start=True, stop=True)
                p["dn"] = dn

            def fin_rec(p):
                rec = asm.tile([1, 1024], F32, name="rec")
                nc.vector.reciprocal_approx_fast(rec[:], p["dn"][:])
                p["rec"] = rec

            def fin_bc(p):
                bc = asm.tile([128, 1024], F32, name="bc")
                nc.gpsimd.partition_broadcast(bc[:], p["rec"][:])
                p["bc"] = bc

            def fin_mul(p):
                ucc, up0, isl = p["ucc"], p["up0"], p["isl"]
                nc.vector.tensor_mul(urt[up0:up0 + 64, ucc, isl],
                                     p["avr"][0:64, :], p["bc"][0:64, :])
                nc.vector.tensor_mul(uit[up0:up0 + 64, ucc, isl],
                                     p["avr"][64:128, :], p["bc"][64:128, :])

            for h in range(HC):
                ucc, up0 = h // 2, (h % 2) * 64
                for iw in range(TW):
                    isl = ts(iw, 1024)
                    av = avp.tile([128, 1024], F32, name="av")
                    esum = asm.tile([128, 1024], BF16, name="esum")
                    for jc in range(TQ):
                        s = mm.tile([128, 1024], F32, name="mmt")
                        for half in range(2):
                            nc.tensor.matmul(
                                s[:, ts(half, 512)],
                                lhsT=qkcat[:, HC + h, ts(jc, 128)],
                                rhs=qkcat[:, h, ts(2 * iw + half, 512)],
                                start=True, stop=True)
                        es = att.tile([128, 1024], BF16, name="es")
                        nc.scalar.activation(es[:], s[:], EXP, scale=0.125)
                        for half in range(2):
                            psl = ts(half, 512)
                            nc.tensor.matmul(av[:, psl],
                                             lhsT=vcat[:, jc, h, :],
                                             rhs=es[:, psl],
                                             start=(jc == 0),
                                             stop=(jc == TQ - 1))
                        if jc == 0:
                            nc.vector.tensor_copy(esum[:], es[:])
                        else:
                            nc.vector.tensor_add(esum[:], esum[:], es[:])
                        if pend is not None:
                            if jc == 1:
                                fin_dn(pend)
                            elif jc == 2:
                                fin_rec(pend)
                            elif jc == 3:
                                fin_bc(pend)
                            elif jc == 5:
                                fin_mul(pend)
                                pend = None
                    avr = asm.tile([128, 1024], BF16, name="avr")
                    nc.vector.tensor_copy(avr[:], av[:])
                    pend = {"esum": esum, "avr": avr,
                            "ucc": ucc, "up0": up0, "isl": isl}
            # flush the final window's finisher
            fin_dn(pend)
            fin_rec(pend)
            fin_bc(pend)
            fin_mul(pend)
            pend = None

            asm.release()
            att.release()
            dnp.release()
            avp.release()
            mm.release()

            # ---------------- output projection ----------------
            with tc.tile_pool(name="ost", bufs=3) as ost, \
                 tc.tile_pool(name="op", bufs=2, space="PSUM") as op:
                for tq in range(TQ):
                    tslq = ts(tq, 128)
                    por = op.tile([128, 1024], F32, name="opa")
                    poi = op.tile([128, 1024], F32, name="opb")
                    for oc in range(2):
                        osl = ts(oc, 512)
                        nc.tensor.matmul(por[:, osl], lhsT=urt[:, 0, tslq],
                                         rhs=ows["r"][:, 0, osl],
                                         start=True, stop=False)
                        nc.tensor.matmul(por[:, osl], lhsT=urt[:, 1, tslq],
                                         rhs=ows["r"][:, 1, osl],
                                         start=False, stop=False)
                        nc.tensor.matmul(por[:, osl], lhsT=uit[:, 0, tslq],
                                         rhs=ows["n"][:, 0, osl],
                                         start=False, stop=False)
                        nc.tensor.matmul(por[:, osl], lhsT=uit[:, 1, tslq],
                                         rhs=ows["n"][:, 1, osl],
                                         start=False, stop=True)
                        nc.tensor.matmul(poi[:, osl], lhsT=urt[:, 0, tslq],
                                         rhs=ows["i"][:, 0, osl],
                                         start=True, stop=False)
                        nc.tensor.matmul(poi[:, osl], lhsT=urt[:, 1, tslq],
                                         rhs=ows["i"][:, 1, osl],
                                         start=False, stop=False)
                        nc.tensor.matmul(poi[:, osl], lhsT=uit[:, 0, tslq],
                                         rhs=ows["r"][:, 0, osl],
                                         start=False, stop=False)
                        nc.tensor.matmul(poi[:, osl], lhsT=uit[:, 1, tslq],
                                         rhs=ows["r"][:, 1, osl],
                                         start=False, stop=True)
                    st = ost.tile([128, 1024], BF16, name="st")
                    sti = ost.tile([128, 1024], BF16, name="sti")
                    for oc in range(2):
                        osl = ts(oc, 512)
                        nc.scalar.copy(st[:, osl], por[:, osl])
                        nc.sync.dma_start(outr_d[tslq, osl], st[:, osl])
                    # outi on the ACT queue: both outputs on SP alone is
                    # bandwidth-bound (8MB) and drains past the last
                    # matmul.
                    for oc in range(2):
                        osl = ts(oc, 512)
                        nc.vector.tensor_copy(sti[:, osl], poi[:, osl])
                        nc.scalar.dma_start(outi_d[tslq, osl], sti[:, osl])

    nc.compile()
    return nc


def _to_bf16_kxm(arr, parts=128):
    """[K, M] fp32 -> [128, K//128, M] bf16 with K split as (chunk, part)."""
    k, m = arr.shape
    out = arr.reshape(k // parts, parts, m).transpose(1, 0, 2)
    return np.ascontiguousarray(out.astype(ml_dtypes.bfloat16))


def _rope_tables():
    inv_freq = 1.0 / (10000.0 ** (np.arange(0, HD, 2, dtype=np.float64) / HD))
    invf64 = np.concatenate([inv_freq, inv_freq])          # [64]
    ang = invf64[:, None] * np.arange(T, dtype=np.float64)[None, :]  # [64, T]
    cos2 = np.tile(np.cos(ang), (2, 1)).astype(ml_dtypes.bfloat16)
    sin2 = np.tile(np.sin(ang), (2, 1)).astype(ml_dtypes.bfloat16)
    return np.ascontiguousarray(cos2), np.ascontiguousarray(sin2)


def kernel(x_real, x_imag, q_wr, q_wi, k_wr, k_wi, v_wr, v_wi, o_wr, o_wi):
    global _COMPILED, LAST_RESULTS
    if _COMPILED is None:
        _COMPILED = _build()
    nc = _COMPILED

    cos2, sin2 = _rope_tables()
    xt = {}
    for b in range(B):
        xt[("r", b)] = _to_bf16_kxm(np.asarray(x_real[b]).T.astype(np.float32))
        xt[("i", b)] = _to_bf16_kxm(np.asarray(x_imag[b]).T.astype(np.float32))

    in_maps = []
    for core in range(NCORE):
        b, g = core // TP, core % TP
        cols = slice(g * C, (g + 1) * C)
        m = {"xrT": xt[("r", b)], "xiT": xt[("i", b)],
             "cos2": cos2, "sin2": sin2}
        for nm, wr_, wi_ in (("wq", q_wr, q_wi), ("wk", k_wr, k_wi)):
            m[f"{nm}_r"] = _to_bf16_kxm(np.asarray(wr_[:, cols]))
            m[f"{nm}_i"] = _to_bf16_kxm(np.asarray(wi_[:, cols]))
            m[f"{nm}_n"] = _to_bf16_kxm(-np.asarray(wi_[:, cols]))
        vr_, vi_ = np.asarray(v_wr[:, cols]), np.asarray(v_wi[:, cols])
        m["wv_a"] = _to_bf16_kxm(np.concatenate([vr_, vi_], axis=1))
        m["wv_b"] = _to_bf16_kxm(np.concatenate([-vi_, vr_], axis=1))
        m["ow_r"] = _to_bf16_kxm(np.asarray(o_wr[cols, :]))
        m["ow_i"] = _to_bf16_kxm(np.asarray(o_wi[cols, :]))
        m["ow_n"] = _to_bf16_kxm(-np.asarray(o_wi[cols, :]))
        in_maps.append(m)

    res = run_bass_kernel_spmd(nc, in_maps, core_ids=list(range(NCORE)))
    LAST_RESULTS = res

    final_r = np.zeros((B, T, D), np.float32)
    final_i = np.zeros((B, T, D), np.float32)
    for core in range(NCORE):
        b = core // TP
        final_r[b] += np.asarray(res.results[core]["out_r"],
                                 dtype=np.float32)
        final_i[b] += np.asarray(res.results[core]["out_i"],
                                 dtype=np.float32)
    return final_r, final_i
